# revision 1
# baseline (speedup 1.0000x reference)
"""Trainium2 Bass kernel for a 2-layer cosine-similarity attention GCN.

Reference math (per (b,h) slice, two chained blocks):
    xn = x / max(||x||_row, eps)
    A  = softmax((xn @ xn^T) / max(alpha, 0.01), axis=-1)
    out = relu((A @ x) @ W^T + x)

Shapes: x [4, 4, 4096, 64] fp32; W [64, 64]. B*H = 16 slices sharded as
2 slices per NeuronCore across 8 cores (fully independent, no collectives).

Kernel strategy (per core, 2 pairs x 2 blocks, all on-chip):
  - logits are cosine sims in [-1,1]*scale -> softmax without max-subtraction:
    P = exp(S*scale) / Z with Z = rowsum. Z is obtained for free by
    augmenting V = x with a ones column and computing U = E @ [x, 1].
  - E tiles are produced directly in [j, i] orientation (j on partitions) so
    the E @ x matmul contracts over partitions; the full softmax matrix is
    never materialized in HBM.
  - division by Z is deferred past the (U @ W^T) matmul (a per-row scale
    commutes with right-multiplication), applied after a PE transpose where
    Z sits on the partition axis.
  - row 1/||x|| uses a fast inverse sqrt (bit trick + 3 Newton steps) on the
    vector engine so the scalar engine only ever runs Exp (no activation
    table switches).
"""

import numpy as np

import concourse.bacc as bacc
import concourse.tile as tile
from concourse import mybir
from concourse.bass_utils import run_bass_kernel_spmd
from concourse.masks import make_identity

F32 = mybir.dt.float32
U32 = mybir.dt.uint32
I32 = mybir.dt.int32
BF16 = mybir.dt.bfloat16
AF = mybir.ActivationFunctionType
ALU = mybir.AluOpType

P = 128
D = 64
N_CORES = 8


def build_nc(scales, n_rows=4096, npairs=2):
    nblocks = len(scales)
    NT = n_rows // P              # row tiles per pair
    CHW = min(1024, n_rows)       # i-chunk width (ACT call width)
    NCH = n_rows // CHW           # chunks
    HALF = 512                    # fp32 PSUM bank width (matmul free dim)
    NH = CHW // HALF

    nc = bacc.Bacc("TRN2", target_bir_lowering=False, debug=False, num_devices=N_CORES)
    xin = nc.dram_tensor("xin", [npairs, n_rows, D], F32, kind="ExternalInput").ap()
    wts = [
        nc.dram_tensor(f"w{i}t", [D, D], F32, kind="ExternalInput").ap()
        for i in range(nblocks)
    ]
    out = nc.dram_tensor("out", [npairs, n_rows, D], F32, kind="ExternalOutput").ap()

    xin_t = xin.rearrange("p (t pp) d -> p pp t d", pp=P)  # [np, 128, NT, 64]
    out_t = out.rearrange("p (t pp) d -> p pp t d", pp=P)

    with tile.TileContext(nc) as tc:
        with (
            tc.tile_pool(name="singles", bufs=1) as singles,
            tc.tile_pool(name="stats", bufs=2) as stats,
            tc.tile_pool(name="tmp", bufs=3) as tmp,
            tc.tile_pool(name="epool", bufs=6) as epool,
            tc.tile_pool(name="fin", bufs=2) as fin,
            tc.tile_pool(name="ps_big", bufs=2, space="PSUM") as ps_big,
            tc.tile_pool(name="ps_u", bufs=2, space="PSUM") as ps_u,
        ):
            ident16 = singles.tile([P, P], BF16, tag="ident16")
            make_identity(nc, ident16[:])
            identf = singles.tile([P, P], F32, tag="identf")
            make_identity(nc, identf[:])

            wf32 = []
            for i in range(nblocks):
                wtmp = singles.tile([D, D], F32, tag=f"wtmp{i}", name=f"wtmp{i}")
                nc.sync.dma_start(wtmp[:], wts[i])
                wf32.append(wtmp)

            # Persistent per (pair, block) state. xnt_all packs pair p's
            # normalized-transposed rows at partitions [64p, 64p+64) so the
            # two pairs' S-matmuls occupy different PE row groups and run
            # concurrently.
            xb = {}    # block input, row-major fp32 [128, NT, 64]
            xb16 = {}  # bf16 copy + ones column (U-matmul stationary operand)
            xnt_all = {}
            for blk in range(nblocks):
                xnt_all[blk] = singles.tile(
                    [P, n_rows], BF16, tag=f"xnt_{blk}", name=f"xnt_{blk}"
                )
            for p in range(npairs):
                for blk in range(nblocks):
                    xb[p, blk] = singles.tile([P, NT, D], F32, tag=f"xb_{p}_{blk}", name=f"xb_{p}_{blk}")
                    xb16[p, blk] = singles.tile(
                        [P, NT, D + 1], BF16, tag=f"xb16_{p}_{blk}", name=f"xb16_{p}_{blk}"
                    )
                    nc.vector.memset(xb16[p, blk][:, :, D : D + 1], 1.0)

            for p in range(npairs):
                nc.sync.dma_start(xb[p, 0][:], xin_t[p])

            MAGIC = 0x5F3759DF

            def prep(p, blk):
                """Row norms -> 1/||x||, normalized bf16 rows, PE-transpose
                into xnt. Also casts xb -> xb16 for block 0 (later blocks get
                the cast fused into the previous block's epilogue)."""
                s_all = stats.tile([P, NT], F32, tag="s_all")
                for b in range(NT):
                    xsl = xb[p, blk][:, b, :]
                    if blk == 0:
                        nc.vector.tensor_copy(xb16[p, blk][:, b, 0:D], xsl)
                    sq = tmp.tile([P, D], F32, tag="sq")
                    nc.vector.tensor_mul(sq[:], xsl, xsl)
                    nc.vector.reduce_sum(
                        s_all[:, b : b + 1], sq[:], axis=mybir.AxisListType.X
                    )
                nc.vector.tensor_scalar_max(s_all[:], s_all[:], 1e-24)
                # rinv = s^-0.5 via fast-inverse-sqrt seed + 3 Newton steps.
                # seed_bits = MAGIC - (bits(s) >> 1), computed overflow-free:
                # t = bits >> 1; v = t - MAGIC (negative int32); seed = ~v + 1
                r = stats.tile([P, NT], F32, tag="rinv")
                s_i = s_all[:].bitcast(I32)
                r_i = r[:].bitcast(I32)
                nc.vector.tensor_scalar(
                    out=r_i, in0=s_i, scalar1=1, scalar2=None,
                    op0=ALU.logical_shift_right,
                )
                nc.vector.tensor_scalar(
                    out=r_i, in0=r_i, scalar1=MAGIC, scalar2=None,
                    op0=ALU.subtract,
                )
                nc.vector.tensor_scalar(
                    out=r_i, in0=r_i, scalar1=-1, scalar2=None,
                    op0=ALU.bitwise_xor,
                )
                nc.vector.tensor_scalar(
                    out=r_i, in0=r_i, scalar1=1, scalar2=None,
                    op0=ALU.add,
                )
                t1 = stats.tile([P, NT], F32, tag="nt1")
                for _ in range(3):
                    nc.vector.tensor_mul(t1[:], r[:], r[:])
                    nc.vector.tensor_mul(t1[:], t1[:], s_all[:])
                    nc.vector.tensor_scalar(
                        out=t1[:],
                        in0=t1[:],
                        scalar1=-0.5,
                        scalar2=1.5,
                        op0=ALU.mult,
                        op1=ALU.add,
                    )
                    nc.vector.tensor_mul(r[:], r[:], t1[:])
                lo = D * p
                for b in range(NT):
                    xn16 = tmp.tile([P, D], BF16, tag="xn16")
                    nc.vector.tensor_scalar_mul(
                        xn16[:], xb[p, blk][:, b, :], r[:, b : b + 1]
                    )
                    pst = ps_big.tile([P, P], BF16, tag="big")
                    nc.tensor.transpose(pst[lo : lo + D, :], xn16[:], ident16[:])
                    nc.vector.tensor_copy(
                        xnt_all[blk][lo : lo + D, b * P : (b + 1) * P],
                        pst[lo : lo + D, :],
                    )

            def process_both(blk, scale, last):
                for a in range(NCH):
                    U = {}
                    for p in range(npairs):
                        U[p] = ps_u.tile([D + 1, CHW], F32, tag="U", name=f"U_{blk}_{a}_{p}")
                    def u_mms(p, b, E_b):
                        for h in range(NH):
                            nc.tensor.matmul(
                                U[p][:, h * HALF : (h + 1) * HALF],
                                lhsT=xb16[p, blk][:, b, :],
                                rhs=E_b[:, h * HALF : (h + 1) * HALF],
                                start=(b == 0),
                                stop=(b == NT - 1),
                            )

                    # U-matmuls run one b-iteration behind the S/exp pipeline
                    # so their E input is already materialized when they reach
                    # the PE queue head (no fine-grained PE stalls -> HAM can
                    # reach full clock).
                    E_prev = None
                    for b in range(NT):
                        E = {}
                        for p in range(npairs):
                            lo = D * p
                            xnt_p = xnt_all[blk][lo : lo + D, :]
                            S = ps_big.tile([P, CHW], F32, tag="big")
                            for h in range(NH):
                                nc.tensor.matmul(
                                    S[:, h * HALF : (h + 1) * HALF],
                                    lhsT=xnt_p[:, b * P : (b + 1) * P],
                                    rhs=xnt_p[
                                        :, a * CHW + h * HALF : a * CHW + (h + 1) * HALF
                                    ],
                                    start=True,
                                    stop=True,
                                )
                            E[p] = epool.tile([P, CHW], BF16, tag="E", name=f"E_{b}_{p}")
                            nc.scalar.activation(E[p][:], S[:], AF.Exp, scale=scale)
                        if E_prev is not None:
                            for p in range(npairs):
                                u_mms(p, b - 1, E_prev[p])
                        E_prev = E
                    for p in range(npairs):
                        u_mms(p, NT - 1, E_prev[p])
                    # chunk epilogue per pair: G = W @ U[:64], pack [G; Z],
                    # transpose, then out = relu(G/Z + x)
                    for p in range(npairs):
                        UTf = fin.tile([D, CHW], F32, tag="UTf")
                        nc.vector.tensor_copy(UTf[:], U[p][0:D, :])
                        G = ps_big.tile([D, CHW], F32, tag="big")
                        for h in range(NH):
                            nc.tensor.matmul(
                                G[:, h * HALF : (h + 1) * HALF],
                                lhsT=wf32[blk][:],
                                rhs=UTf[:, h * HALF : (h + 1) * HALF],
                                start=True,
                                stop=True,
                            )
                        GZ = fin.tile([D + 1, CHW], F32, tag="GZ")
                        nc.vector.tensor_copy(GZ[0:D, :], G[:])
                        nc.vector.tensor_copy(GZ[D : D + 1, :], U[p][D : D + 1, :])
                        for t in range(CHW // P):
                            gi = a * (CHW // P) + t
                            T = ps_big.tile([P, D + 1], F32, tag="big")
                            nc.tensor.transpose(
                                T[:],
                                GZ[:, t * P : (t + 1) * P],
                                identf[0 : D + 1, 0 : D + 1],
                            )
                            rz = tmp.tile([P, 1], F32, tag="rz")
                            nc.vector.reciprocal(rz[:], T[:, D : D + 1])
                            tmpo = tmp.tile([P, D], F32, tag="tmpo")
                            nc.vector.tensor_scalar_mul(tmpo[:], T[:, 0:D], rz[:])
                            nc.vector.tensor_add(tmpo[:], tmpo[:], xb[p, blk][:, gi, :])
                            if not last:
                                dst = xb[p, blk + 1][:, gi, :]
                                nc.vector.tensor_scalar_max(dst, tmpo[:], 0.0)
                                nc.vector.tensor_copy(
                                    xb16[p, blk + 1][:, gi, 0:D], dst
                                )
                            else:
                                oo = tmp.tile([P, D], F32, tag="oo")
                                nc.vector.tensor_scalar_max(oo[:], tmpo[:], 0.0)
                                nc.sync.dma_start(out_t[p][:, gi, :], oo[:])

            for blk in range(nblocks):
                xnt_all[blk] = singles.tile(
                    [P, n_rows], BF16, tag=f"xnt_{blk}", name=f"xnt_{blk}"
                )
            for p in range(npairs):
                for blk in range(nblocks):
                    xb[p, blk] = singles.tile([P, NT, D], F32, tag=f"xb_{p}_{blk}", name=f"xb_{p}_{blk}")
                    xb16[p, blk] = singles.tile(
                        [P, NT, D + 1], BF16, tag=f"xb16_{p}_{blk}", name=f"xb16_{p}_{blk}"
                    )
                    nc.vector.memset(xb16[p, blk][:, :, D : D + 1], 1.0)

            for p in range(npairs):
                nc.sync.dma_start(xb[p, 0][:], xin_t[p])

            MAGIC = 0x5F3759DF

            def prep(p, blk):
                """Row norms -> 1/||x||, normalized bf16 rows, PE-transpose
                into xnt. Also casts xb -> xb16 for block 0 (later blocks get
                the cast fused into the previous block's epilogue)."""
                s_all = stats.tile([P, NT], F32, tag="s_all")
                for b in range(NT):
                    xsl = xb[p, blk][:, b, :]
                    if blk == 0:
                        nc.vector.tensor_copy(xb16[p, blk][:, b, 0:D], xsl)
                    sq = tmp.tile([P, D], F32, tag="sq")
                    nc.vector.tensor_mul(sq[:], xsl, xsl)
                    nc.vector.reduce_sum(
                        s_all[:, b : b + 1], sq[:], axis=mybir.AxisListType.X
                    )
                nc.vector.tensor_scalar_max(s_all[:], s_all[:], 1e-24)
                # rinv = s^-0.5 via fast-inverse-sqrt seed + 3 Newton steps.
                # seed_bits = MAGIC - (bits(s) >> 1), computed overflow-free:
                # t = bits >> 1; v = t - MAGIC (negative int32); seed = ~v + 1
                r = stats.tile([P, NT], F32, tag="rinv")
                s_i = s_all[:].bitcast(I32)
                r_i = r[:].bitcast(I32)
                nc.vector.tensor_scalar(
                    out=r_i, in0=s_i, scalar1=1, scalar2=None,
                    op0=ALU.logical_shift_right,
                )
                nc.vector.tensor_scalar(
                    out=r_i, in0=r_i, scalar1=MAGIC, scalar2=None,
                    op0=ALU.subtract,
                )
                nc.vector.tensor_scalar(
                    out=r_i, in0=r_i, scalar1=-1, scalar2=None,
                    op0=ALU.bitwise_xor,
                )
                nc.vector.tensor_scalar(
                    out=r_i, in0=r_i, scalar1=1, scalar2=None,
                    op0=ALU.add,
                )
                t1 = stats.tile([P, NT], F32, tag="nt1")
                for _ in range(3):
                    nc.vector.tensor_mul(t1[:], r[:], r[:])
                    nc.vector.tensor_mul(t1[:], t1[:], s_all[:])
                    nc.vector.tensor_scalar(
                        out=t1[:],
                        in0=t1[:],
                        scalar1=-0.5,
                        scalar2=1.5,
                        op0=ALU.mult,
                        op1=ALU.add,
                    )
                    nc.vector.tensor_mul(r[:], r[:], t1[:])
                lo = D * p
                for b in range(NT):
                    xn16 = tmp.tile([P, D], BF16, tag="xn16")
                    nc.vector.tensor_scalar_mul(
                        xn16[:], xb[p, blk][:, b, :], r[:, b : b + 1]
                    )
                    pst = ps_big.tile([P, P], BF16, tag="big")
                    nc.tensor.transpose(pst[lo : lo + D, :], xn16[:], ident16[:])
                    nc.vector.tensor_copy(
                        xnt_all[blk][lo : lo + D, b * P : (b + 1) * P],
                        pst[lo : lo + D, :],
                    )

            def process(p, blk, scale, last):
                xnt_c = xnt[p, blk]
                xb16_c = xb16[p, blk]
                for a in range(NCH):
                    U = ps_u.tile([D + 1, CHW], F32, tag="U")
                    for b in range(NT):
                        S = ps_big.tile([P, CHW], F32, tag="big")
                        for h in range(NH):
                            nc.tensor.matmul(
                                S[:, h * HALF : (h + 1) * HALF],
                                lhsT=xnt_c[:, b * P : (b + 1) * P],
                                rhs=xnt_c[
                                    :, a * CHW + h * HALF : a * CHW + (h + 1) * HALF
                                ],
                                start=True,
                                stop=True,
                            )
                        E = epool.tile([P, CHW], BF16, tag="E")
                        nc.scalar.activation(E[:], S[:], AF.Exp, scale=scale)
                        for h in range(NH):
                            nc.tensor.matmul(
                                U[:, h * HALF : (h + 1) * HALF],
                                lhsT=xb16_c[:, b, :],
                                rhs=E[:, h * HALF : (h + 1) * HALF],
                                start=(b == 0),
                                stop=(b == NT - 1),
                            )
                    # chunk epilogue: G = W @ U[:64], pack [G; Z], transpose,
                    # then out = relu(G/Z + x)
                    UTf = fin.tile([D, CHW], F32, tag="UTf")
                    nc.vector.tensor_copy(UTf[:], U[0:D, :])
                    G = ps_big.tile([D, CHW], F32, tag="big")
                    for h in range(NH):
                        nc.tensor.matmul(
                            G[:, h * HALF : (h + 1) * HALF],
                            lhsT=wf32[blk][:],
                            rhs=UTf[:, h * HALF : (h + 1) * HALF],
                            start=True,
                            stop=True,
                        )
                    GZ = fin.tile([D + 1, CHW], F32, tag="GZ")
                    nc.vector.tensor_copy(GZ[0:D, :], G[:])
                    nc.vector.tensor_copy(GZ[D : D + 1, :], U[D : D + 1, :])
                    for t in range(CHW // P):
                        gi = a * (CHW // P) + t
                        T = ps_big.tile([P, D + 1], F32, tag="big")
                        nc.tensor.transpose(
                            T[:],
                            GZ[:, t * P : (t + 1) * P],
                            identf[0 : D + 1, 0 : D + 1],
                        )
                        rz = tmp.tile([P, 1], F32, tag="rz")
                        nc.vector.reciprocal(rz[:], T[:, D : D + 1])
                        tmpo = tmp.tile([P, D], F32, tag="tmpo")
                        nc.vector.tensor_scalar_mul(tmpo[:], T[:, 0:D], rz[:])
                        nc.vector.tensor_add(tmpo[:], tmpo[:], xb[p, blk][:, gi, :])
                        if not last:
                            dst = xb[p, blk + 1][:, gi, :]
                            nc.vector.tensor_scalar_max(dst, tmpo[:], 0.0)
                            nc.vector.tensor_copy(
                                xb16[p, blk + 1][:, gi, 0:D], dst
                            )
                        else:
                            oo = tmp.tile([P, D], F32, tag="oo")
                            nc.vector.tensor_scalar_max(oo[:], tmpo[:], 0.0)
                            nc.sync.dma_start(out_t[p][:, gi, :], oo[:])

            for blk in range(nblocks):
                for p in range(npairs):
                    prep(p, blk)
                process_both(blk, scales[blk], last=(blk == nblocks - 1))

    nc.compile()
    return nc


_CACHE = {}


def _get_nc(scales, n_rows, npairs):
    key = (tuple(scales), n_rows, npairs)
    if key not in _CACHE:
        _CACHE[key] = build_nc(list(scales), n_rows=n_rows, npairs=npairs)
    return _CACHE[key]


def kernel(x, W1, W2, alpha1, alpha2):
    x = np.asarray(x, dtype=np.float32)
    B, H, N, d = x.shape
    assert d == D and (B * H) % N_CORES == 0
    npairs = (B * H) // N_CORES
    s1 = 1.0 / max(float(alpha1), 0.01)
    s2 = 1.0 / max(float(alpha2), 0.01)
    nc = _get_nc((s1, s2), N, npairs)

    xf = np.ascontiguousarray(x.reshape(B * H, N, d))
    w0 = np.ascontiguousarray(np.asarray(W1, dtype=np.float32).T)
    w1 = np.ascontiguousarray(np.asarray(W2, dtype=np.float32).T)
    in_maps = [
        {"xin": xf[npairs * c : npairs * (c + 1)], "w0t": w0, "w1t": w1}
        for c in range(N_CORES)
    ]
    res = run_bass_kernel_spmd(nc, in_maps, core_ids=list(range(N_CORES)))
    outs = np.stack([r["out"] for r in res.results])
    return outs.reshape(B, H, N, d).astype(np.float32)



# revision 6
# speedup vs baseline: 1.5345x; 1.5345x over previous
"""Trainium2 Bass kernel for a 2-layer cosine-similarity attention GCN.

Reference math (per (b,h) slice, two chained blocks):
    xn = x / max(||x||_row, eps)
    A  = softmax((xn @ xn^T) / max(alpha, 0.01), axis=-1)
    out = relu((A @ x) @ W^T + x)

Shapes: x [4, 4, 4096, 64] fp32; W [64, 64]. B*H = 16 slices sharded as
2 slices per NeuronCore across 8 cores (fully independent, no collectives).

Kernel strategy (per core, 2 pairs x 2 blocks, all on-chip):
  - logits are cosine sims in [-1,1]*scale -> softmax without max-subtraction:
    P = exp(S*scale)/Z. E is materialized in fp8e4 (j-on-free orientation via
    the symmetry E^T == E), and U = [x|1]^T E is computed with fp8 DoubleRow
    matmuls (K=256 per instruction, M=80: 64 x-dims + ones col + 15 pad for
    the dual-fp8 LDWEIGHTS step%16 rule). Row 64 of U gives Z for free.
  - exp is split between the ACT engine (native Exp -> fp8 out) and the DVE
    (Schraudolph bit-trick: fp8e4 bits = rne(S*scale*8*log2e + 55.55) written
    as int8), so neither engine is the wall.
  - division by Z is deferred past the W matmul (per-row scale commutes with
    right-multiplication), applied after a PE transpose of [G; Z].
  - chunk epilogues are interleaved into the next chunk's main loop so the
    PE instruction stream stays dense (HAM clock-gate wants sustained busy).
  - row 1/||x|| uses a fast inverse sqrt (bit trick + 3 Newton steps) on the
    vector engine; normalized bf16 rows are produced by ACT Copy-with-scale.
"""

import numpy as np

import concourse.bacc as bacc
import concourse.tile as tile
from concourse import mybir
from concourse.bass_utils import run_bass_kernel_spmd
from concourse.masks import make_identity

F32 = mybir.dt.float32
I8 = mybir.dt.int8
I32 = mybir.dt.int32
BF16 = mybir.dt.bfloat16
FP8 = mybir.dt.float8e4
AF = mybir.ActivationFunctionType
ALU = mybir.AluOpType
DRMODE = mybir.MatmulPerfMode.DoubleRow

P = 128
D = 64
MDR = 80          # DR stationary cols: 64 x | 1 ones | 15 pad (step%16==0)
N_CORES = 8
B_EXP = 55.55     # calibrated Schraudolph offset for fp8e4 (RNE int convert)
ACT_NUM, ACT_DEN = 8, 16   # fraction of exp tiles routed to the ACT engine
INTERLEAVE = False         # run prev-chunk epilogues inside the next chunk loop


def build_nc(scales, n_rows=4096, npairs=2):
    nblocks = len(scales)
    NT = n_rows // P          # 32 row tiles
    NBP = NT // 2             # 16 row-tile pairs (DR K=256)
    CHW = min(1024, n_rows)   # j-chunk width
    NCH = n_rows // CHW
    HALF = 512                # fp32 PSUM bank width
    NH = CHW // HALF
    TPH = HALF // P           # 4 transpose pieces per half

    nc = bacc.Bacc("TRN2", target_bir_lowering=False, debug=False, num_devices=N_CORES)
    xin = nc.dram_tensor("xin", [npairs, n_rows, D], F32, kind="ExternalInput").ap()
    wts = [
        nc.dram_tensor(f"w{i}t", [D, D], F32, kind="ExternalInput").ap()
        for i in range(nblocks)
    ]
    out = nc.dram_tensor("out", [npairs, n_rows, D], F32, kind="ExternalOutput").ap()

    xin_t = xin.rearrange("p (t pp) d -> p pp t d", pp=P)  # [np, 128, NT, 64]
    out_t = out.rearrange("p (t pp) d -> p pp t d", pp=P)

    with tile.TileContext(nc) as tc:
        with (
            tc.tile_pool(name="singles", bufs=1) as singles,
            tc.tile_pool(name="stats", bufs=2) as stats,
            tc.tile_pool(name="tmp", bufs=6) as tmp,
            tc.tile_pool(name="epool", bufs=4) as epool,
            tc.tile_pool(name="fin", bufs=6) as fin,
            tc.tile_pool(name="ps_s", bufs=2, space="PSUM") as ps_s,
            tc.tile_pool(name="ps_uz", bufs=1, space="PSUM") as ps_uz,
        ):
            ident16 = singles.tile([P, P], BF16, tag="ident16")
            make_identity(nc, ident16[:])

            wb16 = []
            for i in range(nblocks):
                wtmp = singles.tile([D, D], F32, tag=f"wtmp{i}", name=f"wtmp{i}")
                nc.sync.dma_start(wtmp[:], wts[i])
                w16 = singles.tile([D, D], BF16, tag=f"w16_{i}", name=f"w16_{i}")
                nc.vector.tensor_copy(w16[:], wtmp[:])
                wb16.append(w16)

            xnt = singles.tile([P, n_rows], BF16, tag="xnt", name="xnt")
            xb = {}
            xb8 = {}
            for p in range(npairs):
                for blk in range(nblocks):
                    xb[p, blk] = singles.tile(
                        [P, NT, D], F32, tag=f"xb_{p}_{blk}", name=f"xb_{p}_{blk}"
                    )
                    xb8[p, blk] = singles.tile(
                        [P, NBP, 2, MDR], FP8, tag=f"xb8_{p}_{blk}", name=f"xb8_{p}_{blk}"
                    )
                    nc.vector.memset(xb8[p, blk][:, :, :, D : D + 1], 1.0)
                    nc.vector.memset(xb8[p, blk][:, :, :, D + 1 : MDR], 0.0)

            for p in range(npairs):
                nc.sync.dma_start(xb[p, 0][:], xin_t[p])

            exp_cnt = [0]

            def emit_exp(dst, src, scale):
                """dst: E2 fp8 slice [128, CHW]; src: S psum [128, CHW] f32."""
                k = exp_cnt[0]
                exp_cnt[0] += 1
                on_act = (k * ACT_NUM) % ACT_DEN < ACT_NUM
                if on_act:
                    nc.scalar.activation(dst, src, AF.Exp, scale=scale)
                else:
                    a_exp = 8.0 * scale / np.log(2.0)
                    nc.vector.tensor_scalar(
                        out=dst.bitcast(I8), in0=src,
                        scalar1=float(a_exp), scalar2=B_EXP,
                        op0=ALU.mult, op1=ALU.add,
                    )

            MAGIC = 0x5F3759DF

            def prep(p, blk):
                """Row norms -> 1/||x||, normalized bf16 rows -> PE transpose
                into xnt; block-0 also casts x -> fp8 DR stationary layout."""
                s_all = stats.tile([P, NT], F32, tag="s_all")
                for b in range(NT):
                    xsl = xb[p, blk][:, b, :]
                    if blk == 0:
                        nc.gpsimd.tensor_copy(
                            xb8[p, blk][:, b // 2, b % 2, 0:D], xsl
                        )
                    scr = tmp.tile([P, D], F32, tag="sq")
                    nc.vector.tensor_mul(scr[:], xsl, xsl)
                    nc.vector.reduce_sum(
                        s_all[:, b : b + 1], scr[:], axis=mybir.AxisListType.X
                    )
                nc.vector.tensor_scalar_max(s_all[:], s_all[:], 1e-24)
                # rinv = s^-0.5 via fast-inverse-sqrt seed + 3 Newton steps.
                r = stats.tile([P, NT], F32, tag="rinv")
                s_i = s_all[:].bitcast(I32)
                r_i = r[:].bitcast(I32)
                nc.vector.tensor_scalar(
                    out=r_i, in0=s_i, scalar1=1, scalar2=None,
                    op0=ALU.logical_shift_right,
                )
                nc.vector.tensor_scalar(
                    out=r_i, in0=r_i, scalar1=MAGIC, scalar2=None, op0=ALU.subtract
                )
                nc.vector.tensor_scalar(
                    out=r_i, in0=r_i, scalar1=-1, scalar2=None, op0=ALU.bitwise_xor
                )
                nc.vector.tensor_scalar(
                    out=r_i, in0=r_i, scalar1=1, scalar2=None, op0=ALU.add
                )
                t1 = stats.tile([P, NT], F32, tag="nt1")
                for _ in range(3):
                    nc.vector.tensor_mul(t1[:], r[:], r[:])
                    nc.vector.tensor_mul(t1[:], t1[:], s_all[:])
                    nc.vector.tensor_scalar(
                        out=t1[:], in0=t1[:], scalar1=-0.5, scalar2=1.5,
                        op0=ALU.mult, op1=ALU.add,
                    )
                    nc.vector.tensor_mul(r[:], r[:], t1[:])
                lo = D * p
                for b in range(NT):
                    xn16 = tmp.tile([P, D], BF16, tag="xn16")
                    nc.vector.tensor_scalar_mul(
                        xn16[:], xb[p, blk][:, b, :], r[:, b : b + 1]
                    )
                    pst = ps_s.tile([P, P], BF16, tag="S")
                    nc.tensor.transpose(pst[lo : lo + D, :], xn16[:], ident16[:])
                    nc.vector.tensor_copy(
                        xnt[lo : lo + D, b * P : (b + 1) * P],
                        pst[lo : lo + D, :],
                    )

            def make_closures(blk, a, uz, last):
                """Epilogue for chunk (blk, a): per (pair, half) prologue
                [UTf copy, G=W@U into the same psum, GZ pack] then 4 transpose
                pieces [T, 1/Z, scale, +x, relu, casts]."""
                closures = []

                def prologue(p, h):
                    def run():
                        u = uz[p, h]
                        utf = fin.tile([D, HALF], BF16, tag="UTf")
                        nc.vector.tensor_copy(utf[:], u[0:D, :])
                        gps = ps_s.tile([P, HALF], F32, tag="S")
                        nc.tensor.matmul(
                            gps[0:D, :], lhsT=wb16[blk][:], rhs=utf[:],
                            start=True, stop=True,
                        )
                        gz = fin.tile([D + 1, HALF], BF16, tag="GZ")
                        nc.vector.tensor_copy(gz[0:D, :], gps[0:D, :])
                        nc.vector.tensor_copy(gz[D : D + 1, :], u[D : D + 1, :])
                        return gz
                    return run

                def tpiece(p, gzref, gi, t):
                    def run():
                        gz = gzref[0]
                        T = ps_s.tile([P, D + 1], BF16, tag="S")
                        nc.tensor.transpose(
                            T[:], gz[:, t * P : (t + 1) * P],
                            ident16[0 : D + 1, 0 : D + 1],
                        )
                        rz = tmp.tile([P, 1], F32, tag="rz")
                        nc.vector.reciprocal(rz[:], T[:, D : D + 1])
                        tmpo = tmp.tile([P, D], F32, tag="tmpo")
                        nc.vector.tensor_scalar_mul(tmpo[:], T[:, 0:D], rz[:])
                        nc.vector.tensor_add(tmpo[:], tmpo[:], xb[p, blk][:, gi, :])
                        if last:
                            oo = tmp.tile([P, D], F32, tag="oo")
                            nc.gpsimd.tensor_scalar_max(oo[:], tmpo[:], 0.0)
                            nc.sync.dma_start(out_t[p][:, gi, :], oo[:])
                        else:
                            dst = xb[p, blk + 1][:, gi, :]
                            nc.gpsimd.tensor_scalar_max(dst, tmpo[:], 0.0)
                            nc.gpsimd.tensor_copy(
                                xb8[p, blk + 1][:, gi // 2, gi % 2, 0:D], dst
                            )
                    return run

                tails = []
                for p in range(npairs):
                    for h in range(NH):
                        gzref = [None]
                        pro = prologue(p, h)

                        def wrap(pro=pro, gzref=gzref):
                            gzref[0] = pro()
                        closures.append(wrap)
                        for t in range(TPH):
                            gi = a * (CHW // P) + h * TPH + t
                            tails.append(tpiece(p, gzref, gi, t))
                return closures + tails

            def emit_u(blk, uz, bp, E2, start, stop):
                for p in range(npairs):
                    for h in range(NH):
                        nc.tensor.matmul(
                            uz[p, h][:],
                            lhsT=xb8[p, blk][:, bp],
                            rhs=E2[p][:, :, h * HALF : (h + 1) * HALF],
                            start=start, stop=stop,
                            perf_mode=DRMODE,
                        )

            ULAG = 2
            pending = []
            for blk in range(nblocks):
                scale = scales[blk]
                for p in range(npairs):
                    prep(p, blk)
                for a in range(NCH):
                    uz = None
                    e2q = []   # (bp, E2 dict) awaiting their U matmuls
                    for bp in range(NBP):
                        E2 = {
                            p: epool.tile([P, 2, CHW], FP8, tag=f"E2_{p}",
                                          name=f"E2_{blk}_{a}_{bp}_{p}")
                            for p in range(npairs)
                        }
                        for g in range(2):
                            b = 2 * bp + g
                            for p in range(npairs):
                                lo = D * p
                                S = ps_s.tile([P, CHW], F32, tag="S")
                                for h in range(NH):
                                    nc.tensor.matmul(
                                        S[:, h * HALF : (h + 1) * HALF],
                                        lhsT=xnt[lo : lo + D, b * P : (b + 1) * P],
                                        rhs=xnt[
                                            lo : lo + D,
                                            a * CHW + h * HALF : a * CHW + (h + 1) * HALF,
                                        ],
                                        start=True, stop=True,
                                    )
                                emit_exp(E2[p][:, g, :], S[:], scale)
                        e2q.append((bp, E2))
                        if bp >= ULAG:
                            if uz is None:
                                uz = {
                                    (p, h): ps_uz.tile(
                                        [MDR, HALF], F32, tag=f"UZ_{p}_{h}",
                                        name=f"UZ_{blk}_{a}_{p}_{h}",
                                    )
                                    for p in range(npairs)
                                    for h in range(NH)
                                }
                            qbp, qE2 = e2q.pop(0)
                            emit_u(blk, uz, qbp, qE2,
                                   start=(qbp == 0), stop=False)
                    # drain leftovers of previous chunk, then remaining U
                    while pending:
                        pending.pop(0)()
                    while e2q:
                        qbp, qE2 = e2q.pop(0)
                        emit_u(blk, uz, qbp, qE2,
                               start=(qbp == 0), stop=(qbp == NBP - 1))
                    pending = make_closures(blk, a, uz, last=(blk == nblocks - 1))
                    if not INTERLEAVE:
                        while pending:
                            pending.pop(0)()
                # block boundary: next prep reads xb[blk+1] -> drain epilogues
                if blk != nblocks - 1:
                    while pending:
                        pending.pop(0)()
            while pending:
                pending.pop(0)()

    nc.compile()
    return nc


_CACHE = {}


def _get_nc(scales, n_rows, npairs):
    key = (tuple(scales), n_rows, npairs)
    if key not in _CACHE:
        _CACHE[key] = build_nc(list(scales), n_rows=n_rows, npairs=npairs)
    return _CACHE[key]


def kernel(x, W1, W2, alpha1, alpha2):
    x = np.asarray(x, dtype=np.float32)
    B, H, N, d = x.shape
    assert d == D and (B * H) % N_CORES == 0
    npairs = (B * H) // N_CORES
    s1 = 1.0 / max(float(alpha1), 0.01)
    s2 = 1.0 / max(float(alpha2), 0.01)
    nc = _get_nc((s1, s2), N, npairs)

    xf = np.ascontiguousarray(x.reshape(B * H, N, d))
    w0 = np.ascontiguousarray(np.asarray(W1, dtype=np.float32).T)
    w1 = np.ascontiguousarray(np.asarray(W2, dtype=np.float32).T)
    in_maps = [
        {"xin": xf[npairs * c : npairs * (c + 1)], "w0t": w0, "w1t": w1}
        for c in range(N_CORES)
    ]
    res = run_bass_kernel_spmd(nc, in_maps, core_ids=list(range(N_CORES)))
    outs = np.stack([r["out"] for r in res.results])
    return outs.reshape(B, H, N, d).astype(np.float32)
